# revision 9
# baseline (speedup 1.0000x reference)
"""Causal self-attention (B=2, S=2048, E=2048, H=16, rope) on 8 TRN2 NeuronCores.

Sharding: batch x head-group. Core c owns batch c//4 and heads
4*(c%4)..4*(c%4)+3: w_qkv rows / w_out columns for its heads; each core
reads only its batch's x (bf16, pre-transposed) and produces a partial
[S, E] bf16 output for its batch; the host sums the 4 partials per batch
(the "all-reduce").

Per-core kernel:
  - xT [E, S] bf16 serves as matmul rhs (Q/K projections -> QT/KT arrive
    transposed [D, S], the layout attention wants) and as lhsT (V
    projection, natural [S, D]).
  - scores are computed transposed: scoresT[k,q] = KT^T @ QT, in panels of
    512 q columns, two k-blocks paired into one [128,1024] PSUM region so
    a single ScalarE exp (softmax scale folded into the activation scale)
    covers both (amortizes the ~352-cycle ACT fixed cost); causal masking
    = per-kb column offsets + one bf16 0/1 mask multiply on the diagonal
    blocks (on GpSimd, which is otherwise idle); A@V accumulates only each
    k-block's causally-valid column range.
  - softmax sums over k: DVE accumulates the exp tiles in bf16 across
    k-blocks (partial column ranges follow causality), then a single
    ones[128,128] matmul per panel reduces over the partition dim with the
    result broadcast across all 128 partitions; reciprocal + multiply fold
    normalization into the y^T PSUM evacuation.
  - attn^T feeds A@V as lhsT directly - no transposes anywhere.
  - rope is applied on DVE during QKV-PSUM evacuation with [D, S] cos /
    signed-sin tables; the half-rotation uses a partition-rolled sin table
    so both multiplies are full-width.
  - startup: ~50 throwaway matmuls warm the PE clock (HAM) during the DMA
    init window; weight/x/constant DMAs are ordered so each first-block
    wave's operands land just before the PE reaches that wave (wq in
    per-wave column chunks, cos/sin in per-token-block chunks).
"""

import math

import numpy as np
import ml_dtypes

import concourse.bass as bass
import concourse.mybir as mybir
import concourse.tile as tile
from concourse import bacc
from concourse.bass_utils import run_bass_kernel_spmd

B, S, E, H, D = 2, 2048, 2048, 16, 128
NCORES = 8
NGRP = 4                    # head groups
HL = H // NGRP              # heads per core = 4
KE = E // 128               # 16 contraction chunks
NB = S // 128               # 16 k/token blocks
NPANEL = S // 512           # 4 q panels
NTB = S // 512              # 4 token blocks for projection
SOFTMAX_SCALE = 1.0 / math.sqrt(D)
BF16 = mybir.dt.bfloat16
F32 = mybir.dt.float32

ROPE_BASE = 10000.0


def _rope_tables():
    inv_freq = 1.0 / (ROPE_BASE ** (np.arange(0, D, 2, dtype=np.float32) / D))
    pos = np.arange(S, dtype=np.float32)
    freqs = np.outer(pos, inv_freq)               # [S, D/2]
    emb = np.concatenate([freqs, freqs], -1)      # [S, D]
    cosT = np.cos(emb).T.astype(np.float32)       # [D, S]
    sinT = np.sin(emb).T.astype(np.float32)
    sinS = sinT.copy()
    sinS[: D // 2] *= -1.0                        # signed: rotate_half sign folded in
    bf = ml_dtypes.bfloat16
    return (np.ascontiguousarray(cosT.astype(bf)),
            np.ascontiguousarray(sinS.astype(bf)))


def _attn_panel(nc, pools, hl, p, q_sb, k_sb, v_sb, y_sb, mask_sb, ones_kk):
    attnp, psum, evacp, accp = pools
    nkb = 4 * p + 4
    yps = psum.tile([128, 512], F32, tag="yps", bufs=2, name=f"yps{hl}{p}")
    acc = accp.tile([128, 512], BF16, tag="acc", bufs=2, name=f"acc{hl}{p}")
    for kb2 in range(nkb // 2):
        kb0, kb1 = 2 * kb2, 2 * kb2 + 1
        # each kb's causally-valid q columns within the panel start at qoff;
        # halves sit at their own qoff inside the pair tile so one shift-free
        # exp covers both (the gap between them is write-only garbage)
        q0 = max(0, kb0 - 4 * p) * 128
        q1 = max(0, kb1 - 4 * p) * 128
        at = attnp.tile([128, 2, 512], BF16, tag="attn", name=f"at{hl}{p}{kb2}")
        ps = psum.tile([128, 2, 512], F32, tag="ps", bufs=2, name=f"sc{hl}{p}{kb2}")
        nc.tensor.matmul(
            ps[:, 0, q0:512],
            lhsT=k_sb[hl][:, kb0 * 128:(kb0 + 1) * 128],
            rhs=q_sb[hl][:, p * 512 + q0:(p + 1) * 512],
            start=True,
            stop=True,
        )
        # second half also starts at q0 (not q1) so the strided exp read
        # below covers only written PSUM; the extra q0..q1 columns are
        # causally invalid and skipped by every downstream read
        nc.tensor.matmul(
            ps[:, 1, q0:512],
            lhsT=k_sb[hl][:, kb1 * 128:(kb1 + 1) * 128],
            rhs=q_sb[hl][:, p * 512 + q0:(p + 1) * 512],
            start=True,
            stop=True,
        )
        nc.scalar.activation(
            at[:, :, q0:512],
            ps[:, :, q0:512],
            mybir.ActivationFunctionType.Exp,
            scale=SOFTMAX_SCALE,
        )
        for half, (kb, qo) in enumerate(((kb0, q0), (kb1, q1))):
            if kb >= 4 * p:  # diagonal block: zero the k>q half
                nc.gpsimd.tensor_mul(
                    at[:, half, qo:qo + 128],
                    at[:, half, qo:qo + 128],
                    mask_sb,
                )
            # softmax denominator: accumulate exp tiles in bf16 on DVE (the
            # partition reduction happens once per panel, below)
            if kb == 0:
                nc.vector.tensor_copy(acc, at[:, 0, :])
            else:
                nc.vector.tensor_add(
                    acc[:, qo:512], acc[:, qo:512], at[:, half, qo:512]
                )
            nc.tensor.matmul(
                yps[:, qo:512],
                lhsT=v_sb[:, kb, hl * D:(hl + 1) * D],
                rhs=at[:, half, qo:512],
                start=(kb == 0),
                stop=(kb == nkb - 1),
            )
    sps = psum.tile([128, 512], F32, tag="ops", bufs=2, name=f"sps{hl}{p}")
    nc.tensor.matmul(sps, lhsT=ones_kk, rhs=acc, start=True, stop=True)
    rb_sb = evacp.tile([128, 512], F32, tag="rb", name=f"rb{hl}{p}")
    nc.vector.reciprocal_approx_fast(out=rb_sb, in_=sps)
    nc.vector.tensor_mul(y_sb[hl][:, p * 512:(p + 1) * 512], yps, rb_sb)


def _emit(nc, tc, xT, wqkvT, w_outT, out, cos_d, sin_d, mask_d):
    from contextlib import ExitStack

    ctx = ExitStack()
    with ctx:
        singles = ctx.enter_context(tc.tile_pool(name="singles", bufs=1))
        xpool = ctx.enter_context(tc.tile_pool(name="xcol", bufs=2))
        persist = ctx.enter_context(tc.tile_pool(name="persist", bufs=1))
        ropet = ctx.enter_context(tc.tile_pool(name="ropet", bufs=2))
        attnp = ctx.enter_context(tc.tile_pool(name="attn", bufs=4))
        evacp = ctx.enter_context(tc.tile_pool(name="evac", bufs=2))
        accp = ctx.enter_context(tc.tile_pool(name="accp", bufs=2))
        outp = ctx.enter_context(tc.tile_pool(name="outp", bufs=4))
        psum = ctx.enter_context(tc.tile_pool(name="psum", bufs=2, space="PSUM"))

        # ---- constant tiles ----
        # wq in per-wave column chunks: wqp[w][ke] holds qk rows 2w..2w+1,
        # wqv[ke] the v rows — separate tiles give region-exact DMA deps so
        # each first-block wave starts as soon as *its* chunk lands
        wqp = [[singles.tile([128, 256], BF16, tag=f"wqp{w}_{ke}", name=f"wqp{w}_{ke}")
                for ke in range(KE)] for w in range(4)]
        wqv = [singles.tile([128, 512], BF16, tag=f"wqv{ke}", name=f"wqv{ke}")
               for ke in range(KE)]
        wo_sb = singles.tile([128, HL, E], BF16, tag="wo")
        cos_sb = [singles.tile([128, 512], BF16, tag=f"cos{tb}", name=f"cos{tb}")
                  for tb in range(NTB)]
        sin_sb = [singles.tile([128, 512], BF16, tag=f"sin{tb}", name=f"sin{tb}")
                  for tb in range(NTB)]
        mask_sb = singles.tile([128, 128], BF16, tag="mask")
        ones_kk = singles.tile([128, 128], BF16, tag="oneskk")
        nc.vector.memset(ones_kk, 1.0)

        # ---- PE warm-up: keep the HAM activity window busy during the DMA
        # init dead time so real matmuls start at full clock ----
        warm = psum.tile([128, 512], F32, tag="ops", bufs=2, name="warm")
        for _ in range(48):
            nc.tensor.matmul(warm[:, 0:128], lhsT=ones_kk, rhs=ones_kk,
                             start=True, stop=True)

        # ---- persistent per-head tensors ----
        q_sb = [persist.tile([128, S], BF16, tag=f"q{h}", name=f"q{h}") for h in range(HL)]
        k_sb = [persist.tile([128, S], BF16, tag=f"k{h}", name=f"k{h}") for h in range(HL)]
        v_sb = persist.tile([128, NB, HL * D], BF16, tag="v", name="v")
        y_sb = [persist.tile([128, S], BF16, tag=f"y{h}", name=f"y{h}") for h in range(HL)]

        pools = (attnp, psum, evacp, accp)

        def proj_block(tb):
            soff = tb * 512
            xc = []
            for ke in range(KE):
                x1 = xpool.tile([128, 512], BF16, tag=f"xc{ke}", name=f"xc{tb}_{ke}")
                if tb == 0:
                    # wave-0 chunk rides along with x so the first chains
                    # start after ~2 small DMAs
                    nc.sync.dma_start(
                        out=wqp[0][ke], in_=wqkvT[ke * 128:(ke + 1) * 128, 0:256]
                    )
                nc.sync.dma_start(
                    out=x1,
                    in_=xT[ke * 128:(ke + 1) * 128, tb * 512:(tb + 1) * 512],
                )
                xc.append(x1)
            if tb == 0:
                # remaining weight chunks + rope tables, ordered to land just
                # before the wave that consumes them
                nc.sync.dma_start(out=cos_sb[0], in_=cos_d[:, 0:512])
                nc.sync.dma_start(out=sin_sb[0], in_=sin_d[:, 0:512])
                for w in range(1, 4):
                    for ke in range(KE):
                        nc.sync.dma_start(
                            out=wqp[w][ke],
                            in_=wqkvT[ke * 128:(ke + 1) * 128, w * 256:(w + 1) * 256],
                        )
                for ke in range(KE):
                    nc.sync.dma_start(
                        out=wqv[ke], in_=wqkvT[ke * 128:(ke + 1) * 128, 1024:1536]
                    )
                nc.sync.dma_start(out=mask_sb, in_=mask_d)
            else:
                nc.sync.dma_start(out=cos_sb[tb], in_=cos_d[:, soff:soff + 512])
                nc.sync.dma_start(out=sin_sb[tb], in_=sin_d[:, soff:soff + 512])
            if tb == 1:
                for hl in range(HL):
                    nc.sync.dma_start(
                        out=wo_sb[:, hl, :], in_=w_outT[hl * 128:(hl + 1) * 128, :]
                    )
            # 12 accumulation chains (8 QK rows + 4 V token-blocks): the PE
            # is in-order, so within a wave each arriving xc chunk feeds the
            # wave's matmuls back to back instead of one chain stalling on
            # the next DMA
            chains = [("qk", rb) for rb in range(2 * HL)] + [
                ("v", tsb) for tsb in range(4)
            ]
            # chains advance in pairs per-ke (pair w matches weight chunk
            # wqp[w]); each pair shares one [128,1024] PSUM tile (same tag
            # as the attention score pairs, so proj+attn fit in 8 banks)
            waves = [chains[i:i + 2] for i in range(0, len(chains), 2)]
            for wv, wave in enumerate(waves):
                pst = psum.tile([128, 2, 512], F32, tag="ps", bufs=2,
                                name=f"p{tb}_{wv}")
                pss = [pst[:, j, :] for j in range(len(wave))]
                for ke in range(KE):
                    for j, (kind, idx) in enumerate(wave):
                        if kind == "qk":
                            nc.tensor.matmul(
                                pss[j],
                                lhsT=wqp[idx // 2][ke][:, (idx % 2) * 128:(idx % 2 + 1) * 128],
                                rhs=xc[ke],
                                start=(ke == 0),
                                stop=(ke == KE - 1),
                            )
                        else:
                            nc.tensor.matmul(
                                pss[j],
                                lhsT=xc[ke][:, idx * 128:(idx + 1) * 128],
                                rhs=wqv[ke],
                                start=(ke == 0),
                                stop=(ke == KE - 1),
                            )
                for j, (kind, idx) in enumerate(wave):
                    ps = pss[j]
                    if kind == "qk":
                        rb = idx
                        # rope: dst = t*cos + swap(t)*sin_signed, bf16 out
                        dst = (q_sb if rb < HL else k_sb)[rb % HL]
                        sl = bass.ds(soff, 512)
                        tsw = ropet.tile([128, 512], F32, tag="tsw", name=f"tsw{tb}{rb}")
                        tco = ropet.tile([128, 512], F32, tag="tco", name=f"tco{tb}{rb}")
                        nc.vector.tensor_mul(tsw[0:64, :], ps[64:128, :], sin_sb[tb][0:64, :])
                        nc.vector.tensor_mul(tsw[64:128, :], ps[0:64, :], sin_sb[tb][64:128, :])
                        nc.vector.tensor_mul(tco, ps, cos_sb[tb])
                        nc.vector.tensor_add(dst[:, sl], tco, tsw)
                    else:
                        blk = (soff // 128) + idx
                        nc.vector.tensor_copy(v_sb[:, blk, :], ps)

        def outproj_panel(p):
            for tkb in range(4 * p, 4 * p + 4):
                tok0 = tkb * 128
                for oc in range(E // 512):
                    ops = psum.tile([128, 512], F32, tag="ops", bufs=2, name=f"o{tkb}{oc}")
                    for hl in range(HL):
                        nc.tensor.matmul(
                            ops,
                            lhsT=y_sb[hl][:, tkb * 128:(tkb + 1) * 128],
                            rhs=wo_sb[:, hl, oc * 512:(oc + 1) * 512],
                            start=(hl == 0),
                            stop=(hl == HL - 1),
                        )
                    ot = outp.tile([128, 512], BF16, tag="ot", name=f"ot{tkb}{oc}")
                    if oc % 2 == 0:
                        nc.scalar.copy(ot, ops)
                    else:
                        nc.vector.tensor_copy(ot, ops)
                    nc.sync.dma_start(
                        out=out[tok0:tok0 + 128, oc * 512:(oc + 1) * 512], in_=ot
                    )

        for tb in range(NTB):
            proj_block(tb)
        for p in range(NPANEL):
            for hl in range(HL):
                _attn_panel(nc, pools, hl, p, q_sb, k_sb, v_sb, y_sb,
                            mask_sb, ones_kk)
            outproj_panel(p)


def build():
    nc = bacc.Bacc("TRN2", target_bir_lowering=False, debug=False)
    xT = nc.dram_tensor("xT", [E, S], BF16, kind="ExternalInput").ap()
    wqkvT = nc.dram_tensor("wqkvT", [E, 3 * HL * D], BF16, kind="ExternalInput").ap()
    w_outT = nc.dram_tensor("w_outT", [HL * D, E], BF16, kind="ExternalInput").ap()
    out = nc.dram_tensor("out", [S, E], BF16, kind="ExternalOutput").ap()

    cosT, sinS = _rope_tables()
    cos_d = nc.inline_tensor(cosT, name="cos_t").ap()
    sin_d = nc.inline_tensor(sinS, name="sin_t").ap()
    # maskT01[k, q] = 1 where k <= q (valid), else 0 — transposed-causal
    mask = np.triu(np.ones((128, 128), np.float32)).astype(ml_dtypes.bfloat16)
    mask_d = nc.inline_tensor(mask, name="maskT01").ap()

    with tile.TileContext(nc) as tc:
        _emit(nc, tc, xT, wqkvT, w_outT, out, cos_d, sin_d, mask_d)
    nc.compile()
    return nc


def core_shard(c):
    """core c -> (batch, head list)."""
    b, g = c // NGRP, c % NGRP
    return b, [HL * g + j for j in range(HL)]


def make_in_maps(x, w_qkv, w_out):
    bf = ml_dtypes.bfloat16
    x2 = np.asarray(x, np.float32).reshape(B, S, E)
    xTs = [np.ascontiguousarray(x2[b].astype(bf).T) for b in range(B)]  # [E, S]
    w_qkv = np.asarray(w_qkv, np.float32)
    w_out = np.asarray(w_out, np.float32)
    in_maps = []
    for c in range(NCORES):
        b, hs = core_shard(c)
        rows = np.concatenate(
            [w_qkv[t * E + h * D:t * E + (h + 1) * D] for t in range(3) for h in hs]
        )                                                           # [1536, E]
        wqkvT = np.ascontiguousarray(rows.astype(bf).T)             # [E, 1536]
        cols = np.concatenate([w_out[:, h * D:(h + 1) * D] for h in hs], axis=1)
        w_outT = np.ascontiguousarray(cols.astype(bf).T)            # [512, E]
        in_maps.append({"xT": xTs[b], "wqkvT": wqkvT, "w_outT": w_outT})
    return in_maps


def gather(results):
    total = np.zeros((B, S, E), np.float32)
    for c, r in enumerate(results):
        b, _ = core_shard(c)
        total[b] += np.asarray(r["out"], np.float32)
    return total


_NC = None


def kernel(x, w_qkv, w_out):
    global _NC
    if _NC is None:
        _NC = build()
    in_maps = make_in_maps(x, w_qkv, w_out)
    res = run_bass_kernel_spmd(_NC, in_maps, core_ids=list(range(NCORES)))
    return gather([r for r in res.results])


# revision 10
# speedup vs baseline: 1.0470x; 1.0470x over previous
"""Causal self-attention (B=2, S=2048, E=2048, H=16, rope) on 8 TRN2 NeuronCores.

Sharding: batch x head-group. Core c owns batch c//4 and heads
4*(c%4)..4*(c%4)+3: w_qkv rows / w_out columns for its heads; each core
reads only its batch's x (bf16, pre-transposed) and produces a partial
[S, E] bf16 output for its batch; the host sums the 4 partials per batch
(the "all-reduce").

Per-core kernel:
  - xT [E, S] bf16 serves as matmul rhs (Q/K projections -> QT/KT arrive
    transposed [D, S], the layout attention wants) and as lhsT (V
    projection, natural [S, D]).
  - scores are computed transposed: scoresT[k,q] = KT^T @ QT, in panels of
    512 q columns, two k-blocks paired into one [128,1024] PSUM region so
    a single ScalarE exp (softmax scale folded into the activation scale)
    covers both (amortizes the ~352-cycle ACT fixed cost); causal masking
    = per-kb column offsets + one bf16 0/1 mask multiply on the diagonal
    blocks (on GpSimd, which is otherwise idle); A@V accumulates only each
    k-block's causally-valid column range.
  - softmax sums over k: DVE accumulates the exp tiles in bf16 across
    k-blocks (partial column ranges follow causality), then a single
    ones[128,128] matmul per panel reduces over the partition dim with the
    result broadcast across all 128 partitions; reciprocal + multiply fold
    normalization into the y^T PSUM evacuation.
  - attn^T feeds A@V as lhsT directly - no transposes anywhere.
  - rope is applied on DVE during QKV-PSUM evacuation with [D, S] cos /
    signed-sin tables; the half-rotation uses a partition-rolled sin table
    so both multiplies are full-width.
  - startup: ~50 throwaway matmuls warm the PE clock (HAM) during the DMA
    init window; weight/x/constant DMAs are ordered so each first-block
    wave's operands land just before the PE reaches that wave (wq in
    per-wave column chunks, cos/sin in per-token-block chunks).
"""

import math

import numpy as np
import ml_dtypes

import concourse.bass as bass
import concourse.mybir as mybir
import concourse.tile as tile
from concourse import bacc
from concourse.bass_utils import run_bass_kernel_spmd

B, S, E, H, D = 2, 2048, 2048, 16, 128
NCORES = 8
NGRP = 4                    # head groups
HL = H // NGRP              # heads per core = 4
KE = E // 128               # 16 contraction chunks
NB = S // 128               # 16 k/token blocks
NPANEL = S // 512           # 4 q panels
NTB = S // 512              # 4 token blocks for projection
SOFTMAX_SCALE = 1.0 / math.sqrt(D)
BF16 = mybir.dt.bfloat16
F32 = mybir.dt.float32

ROPE_BASE = 10000.0


def _rope_tables():
    inv_freq = 1.0 / (ROPE_BASE ** (np.arange(0, D, 2, dtype=np.float32) / D))
    pos = np.arange(S, dtype=np.float32)
    freqs = np.outer(pos, inv_freq)               # [S, D/2]
    emb = np.concatenate([freqs, freqs], -1)      # [S, D]
    cosT = np.cos(emb).T.astype(np.float32)       # [D, S]
    sinT = np.sin(emb).T.astype(np.float32)
    sinS = sinT.copy()
    sinS[: D // 2] *= -1.0                        # signed: rotate_half sign folded in
    bf = ml_dtypes.bfloat16
    return (np.ascontiguousarray(cosT.astype(bf)),
            np.ascontiguousarray(sinS.astype(bf)))


def _attn_panel(nc, pools, hl, p, q_sb, k_sb, v_sb, y_sb, mask_sb, ones_kk):
    attnp, psum, evacp, accp = pools
    nkb = 4 * p + 4
    yps = psum.tile([128, 512], F32, tag="yps", bufs=2, name=f"yps{hl}{p}")
    acc = accp.tile([128, 512], BF16, tag="acc", bufs=2, name=f"acc{hl}{p}")
    for kb2 in range(nkb // 2):
        kb0, kb1 = 2 * kb2, 2 * kb2 + 1
        # each kb's causally-valid q columns within the panel start at qoff;
        # halves sit at their own qoff inside the pair tile so one shift-free
        # exp covers both (the gap between them is write-only garbage)
        q0 = max(0, kb0 - 4 * p) * 128
        q1 = max(0, kb1 - 4 * p) * 128
        at = attnp.tile([128, 2, 512], BF16, tag="attn", name=f"at{hl}{p}{kb2}")
        ps = psum.tile([128, 2, 512], F32, tag="ps", bufs=2, name=f"sc{hl}{p}{kb2}")
        nc.tensor.matmul(
            ps[:, 0, q0:512],
            lhsT=k_sb[hl][:, kb0 * 128:(kb0 + 1) * 128],
            rhs=q_sb[hl][:, p * 512 + q0:(p + 1) * 512],
            start=True,
            stop=True,
        )
        # second half also starts at q0 (not q1) so the strided exp read
        # below covers only written PSUM; the extra q0..q1 columns are
        # causally invalid and skipped by every downstream read
        nc.tensor.matmul(
            ps[:, 1, q0:512],
            lhsT=k_sb[hl][:, kb1 * 128:(kb1 + 1) * 128],
            rhs=q_sb[hl][:, p * 512 + q0:(p + 1) * 512],
            start=True,
            stop=True,
        )
        nc.scalar.activation(
            at[:, :, q0:512],
            ps[:, :, q0:512],
            mybir.ActivationFunctionType.Exp,
            scale=SOFTMAX_SCALE,
        )
        for half, (kb, qo) in enumerate(((kb0, q0), (kb1, q1))):
            if kb >= 4 * p:  # diagonal block: zero the k>q half
                nc.vector.tensor_mul(
                    at[:, half, qo:qo + 128],
                    at[:, half, qo:qo + 128],
                    mask_sb,
                )
            # softmax denominator: accumulate exp tiles in bf16 on DVE (the
            # partition reduction happens once per panel, below)
            if kb == 0:
                nc.vector.tensor_copy(acc, at[:, 0, :])
            else:
                nc.vector.tensor_add(
                    acc[:, qo:512], acc[:, qo:512], at[:, half, qo:512]
                )
            nc.tensor.matmul(
                yps[:, qo:512],
                lhsT=v_sb[:, kb, hl * D:(hl + 1) * D],
                rhs=at[:, half, qo:512],
                start=(kb == 0),
                stop=(kb == nkb - 1),
            )
    sps = psum.tile([128, 512], F32, tag="ops", bufs=2, name=f"sps{hl}{p}")
    nc.tensor.matmul(sps, lhsT=ones_kk, rhs=acc, start=True, stop=True)
    rb_sb = evacp.tile([128, 512], F32, tag="rb", name=f"rb{hl}{p}")
    nc.vector.reciprocal_approx_fast(out=rb_sb, in_=sps)
    nc.vector.tensor_mul(y_sb[hl][:, p * 512:(p + 1) * 512], yps, rb_sb)


def _emit(nc, tc, xT, wqkvT, w_outT, out, cos_d, sin_d, mask_d):
    from contextlib import ExitStack

    ctx = ExitStack()
    with ctx:
        singles = ctx.enter_context(tc.tile_pool(name="singles", bufs=1))
        xpool = ctx.enter_context(tc.tile_pool(name="xcol", bufs=2))
        persist = ctx.enter_context(tc.tile_pool(name="persist", bufs=1))
        ropet = ctx.enter_context(tc.tile_pool(name="ropet", bufs=2))
        attnp = ctx.enter_context(tc.tile_pool(name="attn", bufs=4))
        evacp = ctx.enter_context(tc.tile_pool(name="evac", bufs=2))
        accp = ctx.enter_context(tc.tile_pool(name="accp", bufs=2))
        outp = ctx.enter_context(tc.tile_pool(name="outp", bufs=4))
        psum = ctx.enter_context(tc.tile_pool(name="psum", bufs=2, space="PSUM"))

        # ---- constant tiles ----
        # wq in per-wave column chunks: wqp[w][ke] holds qk rows 2w..2w+1,
        # wqv[ke] the v rows — separate tiles give region-exact DMA deps so
        # each first-block wave starts as soon as *its* chunk lands
        wqp = [[singles.tile([128, 256], BF16, tag=f"wqp{w}_{ke}", name=f"wqp{w}_{ke}")
                for ke in range(KE)] for w in range(4)]
        wqv = [singles.tile([128, 512], BF16, tag=f"wqv{ke}", name=f"wqv{ke}")
               for ke in range(KE)]
        wo_sb = singles.tile([128, HL, E], BF16, tag="wo")
        cos_sb = [singles.tile([128, 512], BF16, tag=f"cos{tb}", name=f"cos{tb}")
                  for tb in range(NTB)]
        sin_sb = [singles.tile([128, 512], BF16, tag=f"sin{tb}", name=f"sin{tb}")
                  for tb in range(NTB)]
        mask_sb = singles.tile([128, 128], BF16, tag="mask")
        ones_kk = singles.tile([128, 128], BF16, tag="oneskk")
        nc.vector.memset(ones_kk, 1.0)

        # ---- PE warm-up: keep the HAM activity window busy during the DMA
        # init dead time so real matmuls start at full clock ----
        warm = psum.tile([128, 512], F32, tag="ops", bufs=2, name="warm")
        for _ in range(48):
            nc.tensor.matmul(warm[:, 0:128], lhsT=ones_kk, rhs=ones_kk,
                             start=True, stop=True)

        # ---- persistent per-head tensors ----
        q_sb = [persist.tile([128, S], BF16, tag=f"q{h}", name=f"q{h}") for h in range(HL)]
        k_sb = [persist.tile([128, S], BF16, tag=f"k{h}", name=f"k{h}") for h in range(HL)]
        v_sb = persist.tile([128, NB, HL * D], BF16, tag="v", name="v")
        y_sb = [persist.tile([128, S], BF16, tag=f"y{h}", name=f"y{h}") for h in range(HL)]

        pools = (attnp, psum, evacp, accp)

        def proj_block(tb):
            soff = tb * 512
            xc = []
            for ke in range(KE):
                x1 = xpool.tile([128, 512], BF16, tag=f"xc{ke}", name=f"xc{tb}_{ke}")
                if tb == 0:
                    # wave-0 chunk rides along with x so the first chains
                    # start after ~2 small DMAs
                    nc.sync.dma_start(
                        out=wqp[0][ke], in_=wqkvT[ke * 128:(ke + 1) * 128, 0:256]
                    )
                nc.sync.dma_start(
                    out=x1,
                    in_=xT[ke * 128:(ke + 1) * 128, tb * 512:(tb + 1) * 512],
                )
                xc.append(x1)
            if tb == 0:
                # remaining weight chunks + rope tables, ordered to land just
                # before the wave that consumes them
                nc.sync.dma_start(out=cos_sb[0], in_=cos_d[:, 0:512])
                nc.sync.dma_start(out=sin_sb[0], in_=sin_d[:, 0:512])
                for w in range(1, 4):
                    for ke in range(KE):
                        nc.sync.dma_start(
                            out=wqp[w][ke],
                            in_=wqkvT[ke * 128:(ke + 1) * 128, w * 256:(w + 1) * 256],
                        )
                for ke in range(KE):
                    nc.sync.dma_start(
                        out=wqv[ke], in_=wqkvT[ke * 128:(ke + 1) * 128, 1024:1536]
                    )
                nc.sync.dma_start(out=mask_sb, in_=mask_d)
            else:
                nc.sync.dma_start(out=cos_sb[tb], in_=cos_d[:, soff:soff + 512])
                nc.sync.dma_start(out=sin_sb[tb], in_=sin_d[:, soff:soff + 512])
            if tb == 1:
                for hl in range(HL):
                    nc.sync.dma_start(
                        out=wo_sb[:, hl, :], in_=w_outT[hl * 128:(hl + 1) * 128, :]
                    )
            # 12 accumulation chains (8 QK rows + 4 V token-blocks): the PE
            # is in-order, so within a wave each arriving xc chunk feeds the
            # wave's matmuls back to back instead of one chain stalling on
            # the next DMA
            chains = [("qk", rb) for rb in range(2 * HL)] + [
                ("v", tsb) for tsb in range(4)
            ]
            # chains advance in pairs per-ke (pair w matches weight chunk
            # wqp[w]); each pair shares one [128,1024] PSUM tile (same tag
            # as the attention score pairs, so proj+attn fit in 8 banks)
            waves = [chains[i:i + 2] for i in range(0, len(chains), 2)]
            for wv, wave in enumerate(waves):
                pst = psum.tile([128, 2, 512], F32, tag="ps", bufs=2,
                                name=f"p{tb}_{wv}")
                pss = [pst[:, j, :] for j in range(len(wave))]
                for ke in range(KE):
                    for j, (kind, idx) in enumerate(wave):
                        if kind == "qk":
                            nc.tensor.matmul(
                                pss[j],
                                lhsT=wqp[idx // 2][ke][:, (idx % 2) * 128:(idx % 2 + 1) * 128],
                                rhs=xc[ke],
                                start=(ke == 0),
                                stop=(ke == KE - 1),
                            )
                        else:
                            nc.tensor.matmul(
                                pss[j],
                                lhsT=xc[ke][:, idx * 128:(idx + 1) * 128],
                                rhs=wqv[ke],
                                start=(ke == 0),
                                stop=(ke == KE - 1),
                            )
                for j, (kind, idx) in enumerate(wave):
                    ps = pss[j]
                    if kind == "qk":
                        rb = idx
                        # rope: dst = t*cos + swap(t)*sin_signed, bf16 out
                        dst = (q_sb if rb < HL else k_sb)[rb % HL]
                        sl = bass.ds(soff, 512)
                        tsw = ropet.tile([128, 512], F32, tag="tsw", name=f"tsw{tb}{rb}")
                        tco = ropet.tile([128, 512], F32, tag="tco", name=f"tco{tb}{rb}")
                        nc.vector.tensor_mul(tsw[0:64, :], ps[64:128, :], sin_sb[tb][0:64, :])
                        nc.vector.tensor_mul(tsw[64:128, :], ps[0:64, :], sin_sb[tb][64:128, :])
                        nc.vector.tensor_mul(tco, ps, cos_sb[tb])
                        nc.vector.tensor_add(dst[:, sl], tco, tsw)
                    else:
                        blk = (soff // 128) + idx
                        nc.vector.tensor_copy(v_sb[:, blk, :], ps)

        def outproj_panel(p):
            for tkb in range(4 * p, 4 * p + 4):
                tok0 = tkb * 128
                for oc in range(E // 512):
                    ops = psum.tile([128, 512], F32, tag="ops", bufs=2, name=f"o{tkb}{oc}")
                    for hl in range(HL):
                        nc.tensor.matmul(
                            ops,
                            lhsT=y_sb[hl][:, tkb * 128:(tkb + 1) * 128],
                            rhs=wo_sb[:, hl, oc * 512:(oc + 1) * 512],
                            start=(hl == 0),
                            stop=(hl == HL - 1),
                        )
                    ot = outp.tile([128, 512], BF16, tag="ot", name=f"ot{tkb}{oc}")
                    if oc % 2 == 0:
                        nc.scalar.copy(ot, ops)
                    else:
                        nc.vector.tensor_copy(ot, ops)
                    nc.sync.dma_start(
                        out=out[tok0:tok0 + 128, oc * 512:(oc + 1) * 512], in_=ot
                    )

        for tb in range(NTB):
            proj_block(tb)
        for p in range(NPANEL):
            for hl in range(HL):
                _attn_panel(nc, pools, hl, p, q_sb, k_sb, v_sb, y_sb,
                            mask_sb, ones_kk)
            outproj_panel(p)


def build():
    nc = bacc.Bacc("TRN2", target_bir_lowering=False, debug=False)
    xT = nc.dram_tensor("xT", [E, S], BF16, kind="ExternalInput").ap()
    wqkvT = nc.dram_tensor("wqkvT", [E, 3 * HL * D], BF16, kind="ExternalInput").ap()
    w_outT = nc.dram_tensor("w_outT", [HL * D, E], BF16, kind="ExternalInput").ap()
    out = nc.dram_tensor("out", [S, E], BF16, kind="ExternalOutput").ap()

    cosT, sinS = _rope_tables()
    cos_d = nc.inline_tensor(cosT, name="cos_t").ap()
    sin_d = nc.inline_tensor(sinS, name="sin_t").ap()
    # maskT01[k, q] = 1 where k <= q (valid), else 0 — transposed-causal
    mask = np.triu(np.ones((128, 128), np.float32)).astype(ml_dtypes.bfloat16)
    mask_d = nc.inline_tensor(mask, name="maskT01").ap()

    with tile.TileContext(nc) as tc:
        _emit(nc, tc, xT, wqkvT, w_outT, out, cos_d, sin_d, mask_d)
    nc.compile()
    return nc


def core_shard(c):
    """core c -> (batch, head list)."""
    b, g = c // NGRP, c % NGRP
    return b, [HL * g + j for j in range(HL)]


def make_in_maps(x, w_qkv, w_out):
    bf = ml_dtypes.bfloat16
    x2 = np.asarray(x, np.float32).reshape(B, S, E)
    xTs = [np.ascontiguousarray(x2[b].astype(bf).T) for b in range(B)]  # [E, S]
    w_qkv = np.asarray(w_qkv, np.float32)
    w_out = np.asarray(w_out, np.float32)
    in_maps = []
    for c in range(NCORES):
        b, hs = core_shard(c)
        rows = np.concatenate(
            [w_qkv[t * E + h * D:t * E + (h + 1) * D] for t in range(3) for h in hs]
        )                                                           # [1536, E]
        wqkvT = np.ascontiguousarray(rows.astype(bf).T)             # [E, 1536]
        cols = np.concatenate([w_out[:, h * D:(h + 1) * D] for h in hs], axis=1)
        w_outT = np.ascontiguousarray(cols.astype(bf).T)            # [512, E]
        in_maps.append({"xT": xTs[b], "wqkvT": wqkvT, "w_outT": w_outT})
    return in_maps


def gather(results):
    total = np.zeros((B, S, E), np.float32)
    for c, r in enumerate(results):
        b, _ = core_shard(c)
        total[b] += np.asarray(r["out"], np.float32)
    return total


_NC = None


def kernel(x, w_qkv, w_out):
    global _NC
    if _NC is None:
        _NC = build()
    in_maps = make_in_maps(x, w_qkv, w_out)
    res = run_bass_kernel_spmd(_NC, in_maps, core_ids=list(range(NCORES)))
    return gather([r for r in res.results])


# revision 11
# speedup vs baseline: 1.0525x; 1.0053x over previous
"""Causal self-attention (B=2, S=2048, E=2048, H=16, rope) on 8 TRN2 NeuronCores.

Sharding: batch x head-group. Core c owns batch c//4 and heads
4*(c%4)..4*(c%4)+3: w_qkv rows / w_out columns for its heads; each core
reads only its batch's x (bf16, pre-transposed) and produces a partial
[S, E] bf16 output for its batch; the host sums the 4 partials per batch
(the "all-reduce").

Per-core kernel:
  - xT [E, S] bf16 serves as matmul rhs (Q/K projections -> QT/KT arrive
    transposed [D, S], the layout attention wants) and as lhsT (V
    projection, natural [S, D]).
  - scores are computed transposed: scoresT[k,q] = KT^T @ QT, in panels of
    512 q columns, two k-blocks paired into one [128,1024] PSUM region so
    a single ScalarE exp (softmax scale folded into the activation scale)
    covers both (amortizes the ~352-cycle ACT fixed cost); causal masking
    = per-kb column offsets + one bf16 0/1 mask multiply on the diagonal
    blocks (on GpSimd, which is otherwise idle); A@V accumulates only each
    k-block's causally-valid column range.
  - softmax sums over k: DVE accumulates the exp tiles in bf16 across
    k-blocks (partial column ranges follow causality), then a single
    ones[128,128] matmul per panel reduces over the partition dim with the
    result broadcast across all 128 partitions; reciprocal + multiply fold
    normalization into the y^T PSUM evacuation.
  - attn^T feeds A@V as lhsT directly - no transposes anywhere.
  - rope is applied on DVE during QKV-PSUM evacuation with [D, S] cos /
    signed-sin tables; the half-rotation uses a partition-rolled sin table
    so both multiplies are full-width.
  - startup: ~50 throwaway matmuls warm the PE clock (HAM) during the DMA
    init window; weight/x/constant DMAs are ordered so each first-block
    wave's operands land just before the PE reaches that wave (wq in
    per-wave column chunks, cos/sin in per-token-block chunks).
"""

import math

import numpy as np
import ml_dtypes

import concourse.bass as bass
import concourse.mybir as mybir
import concourse.tile as tile
from concourse import bacc
from concourse.bass_utils import run_bass_kernel_spmd

B, S, E, H, D = 2, 2048, 2048, 16, 128
NCORES = 8
NGRP = 4                    # head groups
HL = H // NGRP              # heads per core = 4
KE = E // 128               # 16 contraction chunks
NB = S // 128               # 16 k/token blocks
NPANEL = S // 512           # 4 q panels
NTB = S // 512              # 4 token blocks for projection
SOFTMAX_SCALE = 1.0 / math.sqrt(D)
BF16 = mybir.dt.bfloat16
F32 = mybir.dt.float32

ROPE_BASE = 10000.0


def _rope_tables():
    inv_freq = 1.0 / (ROPE_BASE ** (np.arange(0, D, 2, dtype=np.float32) / D))
    pos = np.arange(S, dtype=np.float32)
    freqs = np.outer(pos, inv_freq)               # [S, D/2]
    emb = np.concatenate([freqs, freqs], -1)      # [S, D]
    cosT = np.cos(emb).T.astype(np.float32)       # [D, S]
    sinT = np.sin(emb).T.astype(np.float32)
    sinS = sinT.copy()
    sinS[: D // 2] *= -1.0                        # signed: rotate_half sign folded in
    bf = ml_dtypes.bfloat16
    return (np.ascontiguousarray(cosT.astype(bf)),
            np.ascontiguousarray(sinS.astype(bf)))


def _attn_panel(nc, pools, hl, p, q_sb, k_sb, v_sb, y_sb, mask_sb, ones_kk):
    attnp, psum, evacp, accp = pools
    nkb = 4 * p + 4
    yps = psum.tile([128, 512], F32, tag="yps", bufs=2, name=f"yps{hl}{p}")
    acc = accp.tile([128, 512], BF16, tag="acc", bufs=2, name=f"acc{hl}{p}")
    tiles = []

    def emit_scores(kb2):
        kb0, kb1 = 2 * kb2, 2 * kb2 + 1
        # each kb's causally-valid q columns within the panel start at qoff;
        # halves sit at their own qoff inside the pair tile so one shift-free
        # exp covers both.  The second half also starts at q0 (not q1) so
        # the strided exp read covers only written PSUM; the extra q0..q1
        # columns are causally invalid and skipped by every downstream read
        q0 = max(0, kb0 - 4 * p) * 128
        q1 = max(0, kb1 - 4 * p) * 128
        at = attnp.tile([128, 2, 512], BF16, tag="attn", name=f"at{hl}{p}{kb2}")
        ps = psum.tile([128, 2, 512], F32, tag="ps", bufs=2, name=f"sc{hl}{p}{kb2}")
        for half, kb in ((0, kb0), (1, kb1)):
            nc.tensor.matmul(
                ps[:, half, q0:512],
                lhsT=k_sb[hl][:, kb * 128:(kb + 1) * 128],
                rhs=q_sb[hl][:, p * 512 + q0:(p + 1) * 512],
                start=True,
                stop=True,
            )
        tiles.append((ps, at, kb0, kb1, q0, q1))

    # software-pipelined emission: scores of pair n+1 are queued before the
    # exp(n)-dependent A@V work, so the in-order PE computes them while
    # ScalarE runs exp(n) instead of idling on the cross-engine dependency
    emit_scores(0)
    for kb2 in range(nkb // 2):
        if kb2 + 1 < nkb // 2:
            emit_scores(kb2 + 1)
        ps, at, kb0, kb1, q0, q1 = tiles[kb2]
        nc.scalar.activation(
            at[:, :, q0:512],
            ps[:, :, q0:512],
            mybir.ActivationFunctionType.Exp,
            scale=SOFTMAX_SCALE,
        )
        for half, (kb, qo) in enumerate(((kb0, q0), (kb1, q1))):
            if kb >= 4 * p:  # diagonal block: zero the k>q half
                nc.vector.tensor_mul(
                    at[:, half, qo:qo + 128],
                    at[:, half, qo:qo + 128],
                    mask_sb,
                )
            # softmax denominator: accumulate exp tiles in bf16 on DVE (the
            # partition reduction happens once per panel, below)
            if kb == 0:
                nc.vector.tensor_copy(acc, at[:, 0, :])
            else:
                nc.vector.tensor_add(
                    acc[:, qo:512], acc[:, qo:512], at[:, half, qo:512]
                )
            nc.tensor.matmul(
                yps[:, qo:512],
                lhsT=v_sb[:, kb, hl * D:(hl + 1) * D],
                rhs=at[:, half, qo:512],
                start=(kb == 0),
                stop=(kb == nkb - 1),
            )
    sps = psum.tile([128, 512], F32, tag="ops", bufs=2, name=f"sps{hl}{p}")
    nc.tensor.matmul(sps, lhsT=ones_kk, rhs=acc, start=True, stop=True)
    rb_sb = evacp.tile([128, 512], F32, tag="rb", name=f"rb{hl}{p}")
    nc.vector.reciprocal_approx_fast(out=rb_sb, in_=sps)
    nc.vector.tensor_mul(y_sb[hl][:, p * 512:(p + 1) * 512], yps, rb_sb)


def _emit(nc, tc, xT, wqkvT, w_outT, out, cos_d, sin_d, mask_d):
    from contextlib import ExitStack

    ctx = ExitStack()
    with ctx:
        singles = ctx.enter_context(tc.tile_pool(name="singles", bufs=1))
        xpool = ctx.enter_context(tc.tile_pool(name="xcol", bufs=2))
        persist = ctx.enter_context(tc.tile_pool(name="persist", bufs=1))
        ropet = ctx.enter_context(tc.tile_pool(name="ropet", bufs=2))
        attnp = ctx.enter_context(tc.tile_pool(name="attn", bufs=4))
        evacp = ctx.enter_context(tc.tile_pool(name="evac", bufs=2))
        accp = ctx.enter_context(tc.tile_pool(name="accp", bufs=2))
        outp = ctx.enter_context(tc.tile_pool(name="outp", bufs=4))
        psum = ctx.enter_context(tc.tile_pool(name="psum", bufs=2, space="PSUM"))

        # ---- constant tiles ----
        # wq in per-wave column chunks: wqp[w][ke] holds qk rows 2w..2w+1,
        # wqv[ke] the v rows — separate tiles give region-exact DMA deps so
        # each first-block wave starts as soon as *its* chunk lands
        wqp = [[singles.tile([128, 256], BF16, tag=f"wqp{w}_{ke}", name=f"wqp{w}_{ke}")
                for ke in range(KE)] for w in range(4)]
        wqv = [singles.tile([128, 512], BF16, tag=f"wqv{ke}", name=f"wqv{ke}")
               for ke in range(KE)]
        wo_sb = singles.tile([128, HL, E], BF16, tag="wo")
        cos_sb = [singles.tile([128, 512], BF16, tag=f"cos{tb}", name=f"cos{tb}")
                  for tb in range(NTB)]
        sin_sb = [singles.tile([128, 512], BF16, tag=f"sin{tb}", name=f"sin{tb}")
                  for tb in range(NTB)]
        mask_sb = singles.tile([128, 128], BF16, tag="mask")
        ones_kk = singles.tile([128, 128], BF16, tag="oneskk")
        nc.vector.memset(ones_kk, 1.0)

        # ---- PE warm-up: keep the HAM activity window busy during the DMA
        # init dead time so real matmuls start at full clock ----
        warm = psum.tile([128, 512], F32, tag="ops", bufs=2, name="warm")
        for _ in range(48):
            nc.tensor.matmul(warm[:, 0:128], lhsT=ones_kk, rhs=ones_kk,
                             start=True, stop=True)

        # ---- persistent per-head tensors ----
        q_sb = [persist.tile([128, S], BF16, tag=f"q{h}", name=f"q{h}") for h in range(HL)]
        k_sb = [persist.tile([128, S], BF16, tag=f"k{h}", name=f"k{h}") for h in range(HL)]
        v_sb = persist.tile([128, NB, HL * D], BF16, tag="v", name="v")
        y_sb = [persist.tile([128, S], BF16, tag=f"y{h}", name=f"y{h}") for h in range(HL)]

        pools = (attnp, psum, evacp, accp)

        def proj_block(tb):
            soff = tb * 512
            xc = []
            for ke in range(KE):
                x1 = xpool.tile([128, 512], BF16, tag=f"xc{ke}", name=f"xc{tb}_{ke}")
                if tb == 0:
                    # wave-0 chunk rides along with x so the first chains
                    # start after ~2 small DMAs
                    nc.sync.dma_start(
                        out=wqp[0][ke], in_=wqkvT[ke * 128:(ke + 1) * 128, 0:256]
                    )
                nc.sync.dma_start(
                    out=x1,
                    in_=xT[ke * 128:(ke + 1) * 128, tb * 512:(tb + 1) * 512],
                )
                xc.append(x1)
            if tb == 0:
                # remaining weight chunks + rope tables, ordered to land just
                # before the wave that consumes them
                nc.sync.dma_start(out=cos_sb[0], in_=cos_d[:, 0:512])
                nc.sync.dma_start(out=sin_sb[0], in_=sin_d[:, 0:512])
                for w in range(1, 4):
                    for ke in range(KE):
                        nc.sync.dma_start(
                            out=wqp[w][ke],
                            in_=wqkvT[ke * 128:(ke + 1) * 128, w * 256:(w + 1) * 256],
                        )
                for ke in range(KE):
                    nc.sync.dma_start(
                        out=wqv[ke], in_=wqkvT[ke * 128:(ke + 1) * 128, 1024:1536]
                    )
                nc.sync.dma_start(out=mask_sb, in_=mask_d)
            else:
                nc.sync.dma_start(out=cos_sb[tb], in_=cos_d[:, soff:soff + 512])
                nc.sync.dma_start(out=sin_sb[tb], in_=sin_d[:, soff:soff + 512])
            if tb == 1:
                for hl in range(HL):
                    nc.sync.dma_start(
                        out=wo_sb[:, hl, :], in_=w_outT[hl * 128:(hl + 1) * 128, :]
                    )
            # 12 accumulation chains (8 QK rows + 4 V token-blocks): the PE
            # is in-order, so within a wave each arriving xc chunk feeds the
            # wave's matmuls back to back instead of one chain stalling on
            # the next DMA
            chains = [("qk", rb) for rb in range(2 * HL)] + [
                ("v", tsb) for tsb in range(4)
            ]
            # chains advance in pairs per-ke (pair w matches weight chunk
            # wqp[w]); each pair shares one [128,1024] PSUM tile (same tag
            # as the attention score pairs, so proj+attn fit in 8 banks)
            waves = [chains[i:i + 2] for i in range(0, len(chains), 2)]
            for wv, wave in enumerate(waves):
                pst = psum.tile([128, 2, 512], F32, tag="ps", bufs=2,
                                name=f"p{tb}_{wv}")
                pss = [pst[:, j, :] for j in range(len(wave))]
                for ke in range(KE):
                    for j, (kind, idx) in enumerate(wave):
                        if kind == "qk":
                            nc.tensor.matmul(
                                pss[j],
                                lhsT=wqp[idx // 2][ke][:, (idx % 2) * 128:(idx % 2 + 1) * 128],
                                rhs=xc[ke],
                                start=(ke == 0),
                                stop=(ke == KE - 1),
                            )
                        else:
                            nc.tensor.matmul(
                                pss[j],
                                lhsT=xc[ke][:, idx * 128:(idx + 1) * 128],
                                rhs=wqv[ke],
                                start=(ke == 0),
                                stop=(ke == KE - 1),
                            )
                for j, (kind, idx) in enumerate(wave):
                    ps = pss[j]
                    if kind == "qk":
                        rb = idx
                        # rope: dst = t*cos + swap(t)*sin_signed, bf16 out
                        dst = (q_sb if rb < HL else k_sb)[rb % HL]
                        sl = bass.ds(soff, 512)
                        tsw = ropet.tile([128, 512], F32, tag="tsw", name=f"tsw{tb}{rb}")
                        tco = ropet.tile([128, 512], F32, tag="tco", name=f"tco{tb}{rb}")
                        nc.vector.tensor_mul(tsw[0:64, :], ps[64:128, :], sin_sb[tb][0:64, :])
                        nc.vector.tensor_mul(tsw[64:128, :], ps[0:64, :], sin_sb[tb][64:128, :])
                        nc.vector.tensor_mul(tco, ps, cos_sb[tb])
                        nc.vector.tensor_add(dst[:, sl], tco, tsw)
                    else:
                        blk = (soff // 128) + idx
                        nc.vector.tensor_copy(v_sb[:, blk, :], ps)

        def outproj_panel(p):
            for tkb in range(4 * p, 4 * p + 4):
                tok0 = tkb * 128
                for oc in range(E // 512):
                    ops = psum.tile([128, 512], F32, tag="ops", bufs=2, name=f"o{tkb}{oc}")
                    for hl in range(HL):
                        nc.tensor.matmul(
                            ops,
                            lhsT=y_sb[hl][:, tkb * 128:(tkb + 1) * 128],
                            rhs=wo_sb[:, hl, oc * 512:(oc + 1) * 512],
                            start=(hl == 0),
                            stop=(hl == HL - 1),
                        )
                    ot = outp.tile([128, 512], BF16, tag="ot", name=f"ot{tkb}{oc}")
                    if oc % 2 == 0:
                        nc.scalar.copy(ot, ops)
                    else:
                        nc.vector.tensor_copy(ot, ops)
                    nc.sync.dma_start(
                        out=out[tok0:tok0 + 128, oc * 512:(oc + 1) * 512], in_=ot
                    )

        for tb in range(NTB):
            proj_block(tb)
        for p in range(NPANEL):
            for hl in range(HL):
                _attn_panel(nc, pools, hl, p, q_sb, k_sb, v_sb, y_sb,
                            mask_sb, ones_kk)
            outproj_panel(p)


def build():
    nc = bacc.Bacc("TRN2", target_bir_lowering=False, debug=False)
    xT = nc.dram_tensor("xT", [E, S], BF16, kind="ExternalInput").ap()
    wqkvT = nc.dram_tensor("wqkvT", [E, 3 * HL * D], BF16, kind="ExternalInput").ap()
    w_outT = nc.dram_tensor("w_outT", [HL * D, E], BF16, kind="ExternalInput").ap()
    out = nc.dram_tensor("out", [S, E], BF16, kind="ExternalOutput").ap()

    cosT, sinS = _rope_tables()
    cos_d = nc.inline_tensor(cosT, name="cos_t").ap()
    sin_d = nc.inline_tensor(sinS, name="sin_t").ap()
    # maskT01[k, q] = 1 where k <= q (valid), else 0 — transposed-causal
    mask = np.triu(np.ones((128, 128), np.float32)).astype(ml_dtypes.bfloat16)
    mask_d = nc.inline_tensor(mask, name="maskT01").ap()

    with tile.TileContext(nc) as tc:
        _emit(nc, tc, xT, wqkvT, w_outT, out, cos_d, sin_d, mask_d)
    nc.compile()
    return nc


def core_shard(c):
    """core c -> (batch, head list)."""
    b, g = c // NGRP, c % NGRP
    return b, [HL * g + j for j in range(HL)]


def make_in_maps(x, w_qkv, w_out):
    bf = ml_dtypes.bfloat16
    x2 = np.asarray(x, np.float32).reshape(B, S, E)
    xTs = [np.ascontiguousarray(x2[b].astype(bf).T) for b in range(B)]  # [E, S]
    w_qkv = np.asarray(w_qkv, np.float32)
    w_out = np.asarray(w_out, np.float32)
    in_maps = []
    for c in range(NCORES):
        b, hs = core_shard(c)
        rows = np.concatenate(
            [w_qkv[t * E + h * D:t * E + (h + 1) * D] for t in range(3) for h in hs]
        )                                                           # [1536, E]
        wqkvT = np.ascontiguousarray(rows.astype(bf).T)             # [E, 1536]
        cols = np.concatenate([w_out[:, h * D:(h + 1) * D] for h in hs], axis=1)
        w_outT = np.ascontiguousarray(cols.astype(bf).T)            # [512, E]
        in_maps.append({"xT": xTs[b], "wqkvT": wqkvT, "w_outT": w_outT})
    return in_maps


def gather(results):
    total = np.zeros((B, S, E), np.float32)
    for c, r in enumerate(results):
        b, _ = core_shard(c)
        total[b] += np.asarray(r["out"], np.float32)
    return total


_NC = None


def kernel(x, w_qkv, w_out):
    global _NC
    if _NC is None:
        _NC = build()
    in_maps = make_in_maps(x, w_qkv, w_out)
    res = run_bass_kernel_spmd(_NC, in_maps, core_ids=list(range(NCORES)))
    return gather([r for r in res.results])
